# revision 8
# baseline (speedup 1.0000x reference)
"""Trainium2 Bass kernel for nn_MultiHeadAttention (B=2, S=2048, D=1024, H=16, dk=64).

Sharding: 8 cores = (batch b in {0,1}) x (head group g in {0..3}, 4 heads each).
Key observation: the reference does a RAW reshape (B,H,S,dk) -> (B,S,H*dk)
(mixing head and sequence axes), so output row s' = h*128 + s//16 of X @ WO
depends ONLY on head h.  Core (b,g) therefore produces output rows
[512g, 512(g+1)) of batch b -- a pure concatenation, no collectives.

All matmul operands are bf16 (fp32/f32r matmuls get no HW speedup and block
FWL weight loads). PSUM accumulation stays fp32, output is fp32.

The host pre-permutes Q's sequence axis to j-major order (q~ = (s%16)*128 +
s//16) so the normalized head outputs land CONTIGUOUSLY in the head/seq-mixed
layout the raw reshape needs -- the DVE normalize writes [64,512] packed
instead of a 16-strided scatter (2.9us -> 0.55us per tile), and the scatter
DMA stays the efficient 256B-run form.

Per-core pipeline:
  1. QpT/KpT = W^T @ x^T   -> [heads*dk, S] layouts (host pre-transposes Q,K,V)
     Vp      = x^T.T @ Wv  -> [S, heads*dk] natural layout, augmented with a
               ones column per head (denominator trick).
  2. Per head pair, per q~ block of 512: per k tile (128 kpos):
     scores^T[s_k, q~] single-bank PSUM tiles (2 heads via row tile_position,
     3-deep lookahead), exp via ACT (scale=1/8 fused, fp32 exp needs no max
     subtraction for |score/8| <~ 25) -> bf16 e tiles, P@V with V_aug
     accumulating [65, 512] PSUM where row 64 is the softmax denominator.
  3. recip (DVE) -> broadcast via K=1 outer-product matmul -> normalize on
     DVE into contiguous hr columns.
  4. One scatter DMA per (head, partition parity) assembles X^T tiles;
     WO matmul per head; DMA out.
"""

import sys

try:
    import concourse.bass as bass  # noqa: F401
except ImportError:
    sys.path.insert(0, "/opt/trn_rl_repo")

import ml_dtypes
import numpy as np

import concourse.bacc as bacc
import concourse.tile as tile
from concourse import mybir
from concourse.bass_utils import run_bass_kernel_spmd

BF16 = mybir.dt.bfloat16
F32 = mybir.dt.float32
NP_BF16 = ml_dtypes.bfloat16

B, S, D, H, DK = 2, 2048, 1024, 16, 64
HEADS_PER_CORE = 4
GROUPS = 4
SCALE = 1.0 / 8.0  # 1/sqrt(dk)
E_BUFS = 16

# j-major permutation of the query/sequence axis: column q~ = j*128 + r of
# the permuted qT holds original sequence position s = r*16 + j.
_J = np.arange(S) // 128
_R = np.arange(S) % 128
Q_PERM = _R * 16 + _J  # q~ -> s

_cached_nc = None


def build_nc():
    nc = bacc.Bacc(None, target_bir_lowering=False)
    qT = nc.dram_tensor("qT", [D, S], BF16, kind="ExternalInput")
    kT = nc.dram_tensor("kT", [D, S], BF16, kind="ExternalInput")
    vT = nc.dram_tensor("vT", [D, S], BF16, kind="ExternalInput")
    wq = nc.dram_tensor("wq", [D, 256], BF16, kind="ExternalInput")
    wk = nc.dram_tensor("wk", [D, 256], BF16, kind="ExternalInput")
    wv = nc.dram_tensor("wv", [D, 256], BF16, kind="ExternalInput")
    wo = nc.dram_tensor("wo", [D, D], BF16, kind="ExternalInput")
    out = nc.dram_tensor("out", [512, D], F32, kind="ExternalOutput")

    Exp = mybir.ActivationFunctionType.Exp

    with tile.TileContext(nc) as tc, nc.allow_low_precision(
        reason="bf16 matmul operands with fp32 PSUM accumulation; attention "
        "weights in bf16 average out over 2048 positions"
    ):
        with (
            tc.tile_pool(name="persist", bufs=1) as persist,
            tc.tile_pool(name="hrp", bufs=2) as hrp,
            tc.tile_pool(name="xhp", bufs=2) as xhp,
            tc.tile_pool(name="small", bufs=3) as small,
            tc.tile_pool(name="pvsb", bufs=4) as pvsb,
            tc.tile_pool(name="opool", bufs=2) as opool,
            tc.tile_pool(name="epool", bufs=E_BUFS) as epool,
            tc.tile_pool(name="ps_mix", bufs=2, space="PSUM") as ps_mix,
            tc.tile_pool(name="ps_sc", bufs=4, space="PSUM") as ps_sc,
            tc.tile_pool(name="ps_pv", bufs=2, space="PSUM") as ps_pv,
        ):
            qpT = persist.tile([128, 2, S], BF16, tag="qpT")
            kpT = persist.tile([128, 2, S], BF16, tag="kpT")
            vaug = persist.tile([128, 16, 4, 65], BF16, tag="vaug")
            ones_f32 = persist.tile([128, 1], F32, tag="ones_f32")
            nc.vector.memset(ones_f32, 1.0)
            nc.vector.tensor_copy(
                vaug[:, :, :, 64:65], ones_f32.to_broadcast((128, 16, 4, 1))
            )
            ones = persist.tile([1, 64], BF16, tag="ones")
            nc.vector.tensor_copy(ones, ones_f32[0:1, :].to_broadcast((1, 64)))

            # ---------------- Phase A: projections ----------------
            # Emission order sets scheduler priority: K first (scores lhsT
            # needs all of it), then Q block 0 (first scores rhs), then V
            # (P@V), then remaining Q blocks, then WO weights.
            with (
                tc.tile_pool(name="wqkv", bufs=1) as wqkv,
                tc.tile_pool(name="stream", bufs=3) as stream,
            ):
                wq_sb = wqkv.tile([128, 8, 256], BF16, tag="wq")
                wk_sb = wqkv.tile([128, 8, 256], BF16, tag="wk")
                wv_sb = wqkv.tile([128, 8, 256], BF16, tag="wv")
                for w_dram, w_sb in ((wk, wk_sb), (wq, wq_sb), (wv, wv_sb)):
                    nc.sync.dma_start(
                        out=w_sb, in_=w_dram.rearrange("(t p) n -> p t n", p=128)
                    )

                def proj_qk(x_dram, w_sb, outt, nb):
                    st = stream.tile([128, 8, 512], BF16, tag="acts", name="stq")
                    nc.sync.dma_start(
                        out=st,
                        in_=x_dram.rearrange("(t p) s -> p t s", p=128)[
                            :, :, 512 * nb : 512 * (nb + 1)
                        ],
                    )
                    for m in range(2):
                        ps = ps_mix.tile([128, 512], F32, tag="mix", name="psq")
                        for k in range(8):
                            nc.tensor.matmul(
                                ps,
                                w_sb[:, k, 128 * m : 128 * (m + 1)],
                                st[:, k, :],
                                start=(k == 0),
                                stop=(k == 7),
                            )
                        nc.vector.tensor_copy(
                            outt[:, m, 512 * nb : 512 * (nb + 1)], ps
                        )

                def proj_v(nb):
                    st = stream.tile([128, 8, 512], BF16, tag="acts", name="stv")
                    nc.sync.dma_start(
                        out=st,
                        in_=vT.rearrange("(t p) s -> p t s", p=128)[
                            :, :, 512 * nb : 512 * (nb + 1)
                        ],
                    )
                    for sti in range(4):
                        stt = 4 * nb + sti
                        ps_full = ps_mix.tile([128, 512], F32, tag="mix", name="vps")
                        ps = ps_full[:, :256]
                        for k in range(8):
                            nc.tensor.matmul(
                                ps,
                                st[:, k, 128 * sti : 128 * (sti + 1)],
                                wv_sb[:, k, :],
                                start=(k == 0),
                                stop=(k == 7),
                            )
                        nc.vector.tensor_copy(
                            vaug[:, stt, :, 0:64],
                            ps.rearrange("p (h c) -> p h c", h=4),
                        )

                for nb in range(4):
                    proj_qk(kT, wk_sb, kpT, nb)
                proj_qk(qT, wq_sb, qpT, 0)
                for nb in range(4):
                    proj_v(nb)
                for nb in range(1, 4):
                    proj_qk(qT, wq_sb, qpT, nb)

            # ------------- Phase B/C: attention + output projection -------------
            with tc.tile_pool(name="wop", bufs=1) as wop:
                wo_sb = wop.tile([128, 8, D], BF16, tag="wo")
                nc.sync.dma_start(
                    out=wo_sb, in_=wo.rearrange("(t p) n -> p t n", p=128)
                )

                def attention_block(hp, qb, hr):
                    """scores -> exp -> PV for one (head-pair, q~ block); returns
                    SBUF copies of the PV tiles (pv PSUM banks release fast)."""
                    hA, hB = 2 * hp, 2 * hp + 1
                    pv = {
                        h: ps_pv.tile([65, 512], F32, tag="pv", name=f"pv{h}")
                        for h in (hA, hB)
                    }
                    for kt in range(16):
                        sc = {}
                        e = {}
                        for i, h in enumerate((hA, hB)):
                            sc[h] = ps_sc.tile([128, 512], F32, tag="sc",
                                               name=f"sc{h}")
                            nc.tensor.matmul(
                                sc[h],
                                kpT[64 * i : 64 * (i + 1), hp,
                                    128 * kt : 128 * (kt + 1)],
                                qpT[64 * i : 64 * (i + 1), hp,
                                    512 * qb : 512 * (qb + 1)],
                                start=True,
                                stop=True,
                                tile_position=(64 * i, 0),
                            )
                        for h in (hA, hB):
                            e[h] = epool.tile([128, 512], BF16, tag="e",
                                              name=f"e{h}")
                            nc.scalar.activation(e[h], sc[h], Exp, scale=SCALE)
                        for h in (hA, hB):
                            nc.tensor.matmul(
                                pv[h],
                                vaug[:, kt, h, :],
                                e[h],
                                start=(kt == 0),
                                stop=(kt == 15),
                            )
                    return pv

                def normalize_recips(hp, pv_sb):
                    """DVE half of the normalize: emitted right after the
                    producing block so the reciprocals run during the NEXT
                    block's attention (DVE is in-order)."""
                    rcbs = {}
                    for h in (2 * hp, 2 * hp + 1):
                        rc = small.tile([1, 512], F32, tag="rc", name=f"rc{h}")
                        nc.vector.reciprocal(rc, pv_sb[h][64:65, :])
                        rcb = small.tile([1, 512], BF16, tag="rcb",
                                         name=f"rcb{h}")
                        nc.vector.tensor_copy(rcb, rc)
                        rcbs[h] = rcb
                    return rcbs

                def normalize_apply(hp, qb, hr, pv_sb, rcbs):
                    """PE broadcast + DVE scale; emitted one block late so the
                    PE matmul never stalls the PE stream on the reciprocal."""
                    for h in (2 * hp, 2 * hp + 1):
                        bct = ps_mix.tile([128, 512], F32, tag="mix",
                                          name=f"bct{h}")
                        bc = bct[0:64, :]
                        nc.tensor.matmul(bc, ones, rcbs[h], start=True, stop=True)
                        bc_sb = small.tile([64, 512], F32, tag="bcs",
                                           name=f"bcs{h}")
                        nc.vector.tensor_copy(bc_sb, bc)
                        nc.vector.tensor_mul(
                            hr[h][:, 512 * qb : 512 * (qb + 1)],
                            pv_sb[h][0:64, :], bc_sb,
                        )

                def wo_block(hp, hr):
                    for h in (2 * hp, 2 * hp + 1):
                        xh = xhp.tile([128, 8, 128], BF16, tag="xh", name=f"xh{h}")
                        hv = hr[h].rearrange("p (j r) -> p j r", j=16)
                        for par in range(2):
                            nc.sync.dma_start(
                                out=xh[64 * par : 64 * (par + 1)],
                                in_=hv[:, par::2, :],
                            )
                        for n in range(2):
                            wops = ps_mix.tile([128, 512], F32, tag="mix",
                                               name=f"wops{h}")
                            for t in range(8):
                                nc.tensor.matmul(
                                    wops,
                                    xh[:, t, :],
                                    wo_sb[:, t, 512 * n : 512 * (n + 1)],
                                    start=(t == 0),
                                    stop=(t == 7),
                                )
                            ot = opool.tile([128, 512], F32, tag="o", name=f"ot{h}")
                            nc.vector.tensor_copy(ot, wops)
                            nc.sync.dma_start(
                                out=out[128 * h : 128 * (h + 1),
                                        512 * n : 512 * (n + 1)],
                                in_=ot,
                            )

                hrs = {}
                for hp in range(2):
                    hrs[hp] = {
                        h: hrp.tile([64, 2048], BF16, tag="hr", name=f"hr{h}")
                        for h in (2 * hp, 2 * hp + 1)
                    }
                blocks = [(hp, qb) for hp in range(2) for qb in range(4)]
                pending = None  # (hp, qb, pv_sb, rcbs) awaiting apply
                for idx, (hp, qb) in enumerate(blocks):
                    pv = attention_block(hp, qb, hrs[hp])
                    if pending is not None:
                        p_hp, p_qb, p_pv, p_rcbs = pending
                        normalize_apply(p_hp, p_qb, hrs[p_hp], p_pv, p_rcbs)
                        if p_hp == 0 and p_qb == 3:
                            wo_block(0, hrs[0])
                    # copy PV out of PSUM (frees banks), then queue recips on
                    # DVE so they overlap the next block's attention
                    pv_sb = {}
                    for h in (2 * hp, 2 * hp + 1):
                        pv_sb[h] = pvsb.tile([65, 512], F32, tag="pvs",
                                             name=f"pvs{h}")
                        nc.vector.tensor_copy(pv_sb[h], pv[h])
                    rcbs = normalize_recips(hp, pv_sb)
                    pending = (hp, qb, pv_sb, rcbs)
                p_hp, p_qb, p_pv, p_rcbs = pending
                normalize_apply(p_hp, p_qb, hrs[p_hp], p_pv, p_rcbs)
                wo_block(1, hrs[1])

    nc.finalize()
    return nc


def make_in_maps(Q, K, V, WQ, WK, WV, WO):
    in_maps = []
    wo_full = np.ascontiguousarray(WO.astype(NP_BF16))
    for b in range(B):
        # qT columns permuted to j-major order (see Q_PERM)
        qTb = np.ascontiguousarray(Q[b].T[:, Q_PERM].astype(NP_BF16))
        kTb = np.ascontiguousarray(K[b].T.astype(NP_BF16))
        vTb = np.ascontiguousarray(V[b].T.astype(NP_BF16))
        for g in range(GROUPS):
            hs = slice(4 * g, 4 * g + 4)
            # [4, D, dk] -> [D, 4*dk]
            wqc = np.ascontiguousarray(
                WQ[hs].transpose(1, 0, 2).reshape(D, 256).astype(NP_BF16)
            )
            wkc = np.ascontiguousarray(
                WK[hs].transpose(1, 0, 2).reshape(D, 256).astype(NP_BF16)
            )
            wvc = np.ascontiguousarray(
                WV[hs].transpose(1, 0, 2).reshape(D, 256).astype(NP_BF16)
            )
            in_maps.append(
                {"qT": qTb, "kT": kTb, "vT": vTb,
                 "wq": wqc, "wk": wkc, "wv": wvc, "wo": wo_full}
            )
    return in_maps


def run(inputs, **run_kwargs):
    global _cached_nc
    if _cached_nc is None:
        _cached_nc = build_nc()
    in_maps = make_in_maps(**inputs)
    res = run_bass_kernel_spmd(
        _cached_nc, in_maps, core_ids=list(range(8)), **run_kwargs
    )
    full = np.zeros((B, S, D), np.float32)
    for b in range(B):
        for g in range(GROUPS):
            full[b, 512 * g : 512 * (g + 1), :] = res.results[4 * b + g]["out"]
    return full, res


def kernel(**inputs):
    full, _ = run(inputs)
    return full


if __name__ == "__main__":
    rng = np.random.default_rng(0)
    inputs = {
        "Q": rng.standard_normal((B, S, D)).astype(np.float32),
        "K": rng.standard_normal((B, S, D)).astype(np.float32),
        "V": rng.standard_normal((B, S, D)).astype(np.float32),
        "WQ": (rng.uniform(-0.1, 0.1, (H, D, DK))).astype(np.float32),
        "WK": (rng.uniform(-0.1, 0.1, (H, D, DK))).astype(np.float32),
        "WV": (rng.uniform(-0.1, 0.1, (H, D, DK))).astype(np.float32),
        "WO": (rng.uniform(-0.1, 0.1, (H * DK, D))).astype(np.float32),
    }
    out = kernel(**inputs)
    print("kernel out", out.shape, out.dtype, float(np.abs(out).max()))


# revision 9
# speedup vs baseline: 1.0136x; 1.0136x over previous
"""Trainium2 Bass kernel for nn_MultiHeadAttention (B=2, S=2048, D=1024, H=16, dk=64).

Sharding: 8 cores = (batch b in {0,1}) x (head group g in {0..3}, 4 heads each).
Key observation: the reference does a RAW reshape (B,H,S,dk) -> (B,S,H*dk)
(mixing head and sequence axes), so output row s' = h*128 + s//16 of X @ WO
depends ONLY on head h.  Core (b,g) therefore produces output rows
[512g, 512(g+1)) of batch b -- a pure concatenation, no collectives.

All matmul operands are bf16 (fp32/f32r matmuls get no HW speedup and block
FWL weight loads). PSUM accumulation stays fp32, output is fp32.

The host pre-permutes Q's sequence axis to j-major order (q~ = (s%16)*128 +
s//16) so the normalized head outputs land CONTIGUOUSLY in the head/seq-mixed
layout the raw reshape needs -- the DVE normalize writes [64,512] packed
instead of a 16-strided scatter (2.9us -> 0.55us per tile), and the scatter
DMA stays the efficient 256B-run form.

Per-core pipeline:
  1. QpT/KpT = W^T @ x^T   -> [heads*dk, S] layouts (host pre-transposes Q,K,V)
     Vp      = x^T.T @ Wv  -> [S, heads*dk] natural layout, augmented with a
               ones column per head (denominator trick).
  2. Per head pair, per q~ block of 512: per k tile (128 kpos):
     scores^T[s_k, q~] single-bank PSUM tiles (2 heads via row tile_position,
     3-deep lookahead), exp via ACT (scale=1/8 fused, fp32 exp needs no max
     subtraction for |score/8| <~ 25) -> bf16 e tiles, P@V with V_aug
     accumulating [65, 512] PSUM where row 64 is the softmax denominator.
  3. recip (DVE) -> broadcast via K=1 outer-product matmul -> normalize on
     DVE into contiguous hr columns.
  4. One scatter DMA per (head, partition parity) assembles X^T tiles;
     WO matmul per head; DMA out.
"""

import sys

try:
    import concourse.bass as bass  # noqa: F401
except ImportError:
    sys.path.insert(0, "/opt/trn_rl_repo")

import ml_dtypes
import numpy as np

import concourse.bacc as bacc
import concourse.tile as tile
from concourse import mybir
from concourse.bass_utils import run_bass_kernel_spmd

BF16 = mybir.dt.bfloat16
F32 = mybir.dt.float32
NP_BF16 = ml_dtypes.bfloat16

B, S, D, H, DK = 2, 2048, 1024, 16, 64
HEADS_PER_CORE = 4
GROUPS = 4
SCALE = 1.0 / 8.0  # 1/sqrt(dk)
E_BUFS = 16

# j-major permutation of the query/sequence axis: column q~ = j*128 + r of
# the permuted qT holds original sequence position s = r*16 + j.
_J = np.arange(S) // 128
_R = np.arange(S) % 128
Q_PERM = _R * 16 + _J  # q~ -> s

_cached_nc = None


def build_nc():
    nc = bacc.Bacc(None, target_bir_lowering=False)
    qT = nc.dram_tensor("qT", [D, S], BF16, kind="ExternalInput")
    kT = nc.dram_tensor("kT", [D, S], BF16, kind="ExternalInput")
    vT = nc.dram_tensor("vT", [D, S], BF16, kind="ExternalInput")
    wq = nc.dram_tensor("wq", [D, 256], BF16, kind="ExternalInput")
    wk = nc.dram_tensor("wk", [D, 256], BF16, kind="ExternalInput")
    wv = nc.dram_tensor("wv", [D, 256], BF16, kind="ExternalInput")
    wo = nc.dram_tensor("wo", [D, D], BF16, kind="ExternalInput")
    out = nc.dram_tensor("out", [512, D], F32, kind="ExternalOutput")

    Exp = mybir.ActivationFunctionType.Exp

    with tile.TileContext(nc) as tc, nc.allow_low_precision(
        reason="bf16 matmul operands with fp32 PSUM accumulation; attention "
        "weights in bf16 average out over 2048 positions"
    ):
        with (
            tc.tile_pool(name="persist", bufs=1) as persist,
            tc.tile_pool(name="hrp", bufs=2) as hrp,
            tc.tile_pool(name="xhp", bufs=2) as xhp,
            tc.tile_pool(name="small", bufs=3) as small,
            tc.tile_pool(name="pvsb", bufs=4) as pvsb,
            tc.tile_pool(name="opool", bufs=2) as opool,
            tc.tile_pool(name="epool", bufs=E_BUFS) as epool,
            tc.tile_pool(name="ps_mix", bufs=2, space="PSUM") as ps_mix,
            tc.tile_pool(name="ps_sc", bufs=4, space="PSUM") as ps_sc,
            tc.tile_pool(name="ps_pv", bufs=2, space="PSUM") as ps_pv,
        ):
            qpT = persist.tile([128, 2, S], BF16, tag="qpT")
            kpT = persist.tile([128, 2, S], BF16, tag="kpT")
            vaug = persist.tile([128, 16, 4, 65], BF16, tag="vaug")
            ones_f32 = persist.tile([128, 1], F32, tag="ones_f32")
            nc.vector.memset(ones_f32, 1.0)
            nc.vector.tensor_copy(
                vaug[:, :, :, 64:65], ones_f32.to_broadcast((128, 16, 4, 1))
            )
            ones = persist.tile([1, 64], BF16, tag="ones")
            nc.vector.tensor_copy(ones, ones_f32[0:1, :].to_broadcast((1, 64)))

            # ---------------- Phase A: projections ----------------
            # Emission order sets scheduler priority: K first (scores lhsT
            # needs all of it), then Q block 0 (first scores rhs), then V
            # (P@V), then remaining Q blocks, then WO weights.
            with (
                tc.tile_pool(name="wqkv", bufs=1) as wqkv,
                tc.tile_pool(name="stream", bufs=3) as stream,
            ):
                wq_sb = wqkv.tile([128, 8, 256], BF16, tag="wq")
                wk_sb = wqkv.tile([128, 8, 256], BF16, tag="wk")
                wv_sb = wqkv.tile([128, 8, 256], BF16, tag="wv")
                for w_dram, w_sb in ((wk, wk_sb), (wq, wq_sb), (wv, wv_sb)):
                    nc.sync.dma_start(
                        out=w_sb, in_=w_dram.rearrange("(t p) n -> p t n", p=128)
                    )

                def proj_qk(x_dram, w_sb, outt, nb):
                    st = stream.tile([128, 8, 512], BF16, tag="acts", name="stq")
                    nc.sync.dma_start(
                        out=st,
                        in_=x_dram.rearrange("(t p) s -> p t s", p=128)[
                            :, :, 512 * nb : 512 * (nb + 1)
                        ],
                    )
                    for m in range(2):
                        ps = ps_mix.tile([128, 512], F32, tag="mix", name="psq")
                        for k in range(8):
                            nc.tensor.matmul(
                                ps,
                                w_sb[:, k, 128 * m : 128 * (m + 1)],
                                st[:, k, :],
                                start=(k == 0),
                                stop=(k == 7),
                            )
                        nc.vector.tensor_copy(
                            outt[:, m, 512 * nb : 512 * (nb + 1)], ps
                        )

                def proj_v(nb):
                    st = stream.tile([128, 8, 512], BF16, tag="acts", name="stv")
                    nc.sync.dma_start(
                        out=st,
                        in_=vT.rearrange("(t p) s -> p t s", p=128)[
                            :, :, 512 * nb : 512 * (nb + 1)
                        ],
                    )
                    for sti in range(4):
                        stt = 4 * nb + sti
                        ps_full = ps_mix.tile([128, 512], F32, tag="mix", name="vps")
                        ps = ps_full[:, :256]
                        for k in range(8):
                            nc.tensor.matmul(
                                ps,
                                st[:, k, 128 * sti : 128 * (sti + 1)],
                                wv_sb[:, k, :],
                                start=(k == 0),
                                stop=(k == 7),
                            )
                        nc.vector.tensor_copy(
                            vaug[:, stt, :, 0:64],
                            ps.rearrange("p (h c) -> p h c", h=4),
                        )

                for nb in range(4):
                    proj_qk(kT, wk_sb, kpT, nb)
                proj_qk(qT, wq_sb, qpT, 0)
                for nb in range(4):
                    proj_v(nb)
                for nb in range(1, 4):
                    proj_qk(qT, wq_sb, qpT, nb)

            # ------------- Phase B/C: attention + output projection -------------
            with tc.tile_pool(name="wop", bufs=1) as wop:
                wo_sb = wop.tile([128, 8, D], BF16, tag="wo")
                nc.sync.dma_start(
                    out=wo_sb, in_=wo.rearrange("(t p) n -> p t n", p=128)
                )

                def attention_block(hp, qb, hr):
                    """scores -> exp -> PV for one (head-pair, q~ block); returns
                    SBUF copies of the PV tiles (pv PSUM banks release fast)."""
                    hA, hB = 2 * hp, 2 * hp + 1
                    pv = {
                        h: ps_pv.tile([65, 512], F32, tag="pv", name=f"pv{h}")
                        for h in (hA, hB)
                    }
                    for kt in range(16):
                        sc = {}
                        e = {}
                        for i, h in enumerate((hA, hB)):
                            sc[h] = ps_sc.tile([128, 512], F32, tag="sc",
                                               name=f"sc{h}")
                            nc.tensor.matmul(
                                sc[h],
                                kpT[64 * i : 64 * (i + 1), hp,
                                    128 * kt : 128 * (kt + 1)],
                                qpT[64 * i : 64 * (i + 1), hp,
                                    512 * qb : 512 * (qb + 1)],
                                start=True,
                                stop=True,
                                tile_position=(64 * i, 0),
                            )
                        for h in (hA, hB):
                            e[h] = epool.tile([128, 512], BF16, tag="e",
                                              name=f"e{h}")
                            nc.scalar.activation(e[h], sc[h], Exp, scale=SCALE)
                        for h in (hA, hB):
                            nc.tensor.matmul(
                                pv[h],
                                vaug[:, kt, h, :],
                                e[h],
                                start=(kt == 0),
                                stop=(kt == 15),
                            )
                    return pv

                def normalize_recips(hp, pv_sb):
                    """DVE half of the normalize: emitted right after the
                    producing block so the reciprocals run during the NEXT
                    block's attention (DVE is in-order)."""
                    rcbs = {}
                    for h in (2 * hp, 2 * hp + 1):
                        rc = small.tile([1, 512], F32, tag="rc", name=f"rc{h}")
                        nc.vector.reciprocal(rc, pv_sb[h][64:65, :])
                        rcb = small.tile([1, 512], BF16, tag="rcb",
                                         name=f"rcb{h}")
                        nc.vector.tensor_copy(rcb, rc)
                        rcbs[h] = rcb
                    return rcbs

                def normalize_apply(hp, qb, hr, pv_sb, rcbs):
                    """PE broadcast + DVE scale; emitted one block late so the
                    PE matmul never stalls the PE stream on the reciprocal."""
                    for h in (2 * hp, 2 * hp + 1):
                        bct = ps_mix.tile([128, 512], F32, tag="mix",
                                          name=f"bct{h}")
                        bc = bct[0:64, :]
                        nc.tensor.matmul(bc, ones, rcbs[h], start=True, stop=True)
                        bc_sb = small.tile([64, 512], F32, tag="bcs",
                                           name=f"bcs{h}")
                        nc.scalar.copy(bc_sb, bc)
                        nc.vector.tensor_mul(
                            hr[h][:, 512 * qb : 512 * (qb + 1)],
                            pv_sb[h][0:64, :], bc_sb,
                        )

                def wo_block(hp, hr):
                    for h in (2 * hp, 2 * hp + 1):
                        xh = xhp.tile([128, 8, 128], BF16, tag="xh", name=f"xh{h}")
                        hv = hr[h].rearrange("p (j r) -> p j r", j=16)
                        for par in range(2):
                            nc.sync.dma_start(
                                out=xh[64 * par : 64 * (par + 1)],
                                in_=hv[:, par::2, :],
                            )
                        for n in range(2):
                            wops = ps_mix.tile([128, 512], F32, tag="mix",
                                               name=f"wops{h}")
                            for t in range(8):
                                nc.tensor.matmul(
                                    wops,
                                    xh[:, t, :],
                                    wo_sb[:, t, 512 * n : 512 * (n + 1)],
                                    start=(t == 0),
                                    stop=(t == 7),
                                )
                            ot = opool.tile([128, 512], F32, tag="o", name=f"ot{h}")
                            nc.vector.tensor_copy(ot, wops)
                            nc.sync.dma_start(
                                out=out[128 * h : 128 * (h + 1),
                                        512 * n : 512 * (n + 1)],
                                in_=ot,
                            )

                hrs = {}
                for hp in range(2):
                    hrs[hp] = {
                        h: hrp.tile([64, 2048], BF16, tag="hr", name=f"hr{h}")
                        for h in (2 * hp, 2 * hp + 1)
                    }
                blocks = [(hp, qb) for hp in range(2) for qb in range(4)]
                pending = None  # (hp, qb, pv_sb, rcbs) awaiting apply
                for idx, (hp, qb) in enumerate(blocks):
                    pv = attention_block(hp, qb, hrs[hp])
                    # copy PV out of PSUM (frees banks), then queue recips on
                    # DVE: both run at this block's end / next block's start,
                    # BEFORE the deferred apply ops in every engine stream
                    pv_sb = {}
                    for h in (2 * hp, 2 * hp + 1):
                        pv_sb[h] = pvsb.tile([65, 512], F32, tag="pvs",
                                             name=f"pvs{h}")
                        nc.vector.tensor_copy(pv_sb[h], pv[h])
                    rcbs = normalize_recips(hp, pv_sb)
                    if pending is not None:
                        p_hp, p_qb, p_pv, p_rcbs = pending
                        normalize_apply(p_hp, p_qb, hrs[p_hp], p_pv, p_rcbs)
                        if p_hp == 0 and p_qb == 3:
                            wo_block(0, hrs[0])
                    pending = (hp, qb, pv_sb, rcbs)
                p_hp, p_qb, p_pv, p_rcbs = pending
                normalize_apply(p_hp, p_qb, hrs[p_hp], p_pv, p_rcbs)
                wo_block(1, hrs[1])

    nc.finalize()
    return nc


def make_in_maps(Q, K, V, WQ, WK, WV, WO):
    in_maps = []
    wo_full = np.ascontiguousarray(WO.astype(NP_BF16))
    for b in range(B):
        # qT columns permuted to j-major order (see Q_PERM)
        qTb = np.ascontiguousarray(Q[b].T[:, Q_PERM].astype(NP_BF16))
        kTb = np.ascontiguousarray(K[b].T.astype(NP_BF16))
        vTb = np.ascontiguousarray(V[b].T.astype(NP_BF16))
        for g in range(GROUPS):
            hs = slice(4 * g, 4 * g + 4)
            # [4, D, dk] -> [D, 4*dk]
            wqc = np.ascontiguousarray(
                WQ[hs].transpose(1, 0, 2).reshape(D, 256).astype(NP_BF16)
            )
            wkc = np.ascontiguousarray(
                WK[hs].transpose(1, 0, 2).reshape(D, 256).astype(NP_BF16)
            )
            wvc = np.ascontiguousarray(
                WV[hs].transpose(1, 0, 2).reshape(D, 256).astype(NP_BF16)
            )
            in_maps.append(
                {"qT": qTb, "kT": kTb, "vT": vTb,
                 "wq": wqc, "wk": wkc, "wv": wvc, "wo": wo_full}
            )
    return in_maps


def run(inputs, **run_kwargs):
    global _cached_nc
    if _cached_nc is None:
        _cached_nc = build_nc()
    in_maps = make_in_maps(**inputs)
    res = run_bass_kernel_spmd(
        _cached_nc, in_maps, core_ids=list(range(8)), **run_kwargs
    )
    full = np.zeros((B, S, D), np.float32)
    for b in range(B):
        for g in range(GROUPS):
            full[b, 512 * g : 512 * (g + 1), :] = res.results[4 * b + g]["out"]
    return full, res


def kernel(**inputs):
    full, _ = run(inputs)
    return full


if __name__ == "__main__":
    rng = np.random.default_rng(0)
    inputs = {
        "Q": rng.standard_normal((B, S, D)).astype(np.float32),
        "K": rng.standard_normal((B, S, D)).astype(np.float32),
        "V": rng.standard_normal((B, S, D)).astype(np.float32),
        "WQ": (rng.uniform(-0.1, 0.1, (H, D, DK))).astype(np.float32),
        "WK": (rng.uniform(-0.1, 0.1, (H, D, DK))).astype(np.float32),
        "WV": (rng.uniform(-0.1, 0.1, (H, D, DK))).astype(np.float32),
        "WO": (rng.uniform(-0.1, 0.1, (H * DK, D))).astype(np.float32),
    }
    out = kernel(**inputs)
    print("kernel out", out.shape, out.dtype, float(np.abs(out).max()))


# revision 10
# speedup vs baseline: 1.4443x; 1.4249x over previous
"""Trainium2 Bass kernel for nn_MultiHeadAttention (B=2, S=2048, D=1024, H=16, dk=64).

Sharding: 8 cores = (batch b in {0,1}) x (head group g in {0..3}, 4 heads each).
Key observation: the reference does a RAW reshape (B,H,S,dk) -> (B,S,H*dk)
(mixing head and sequence axes), so output row s' = h*128 + s//16 of X @ WO
depends ONLY on head h.  Core (b,g) therefore produces output rows
[512g, 512(g+1)) of batch b -- a pure concatenation, no collectives.

All matmul operands are bf16 (fp32/f32r matmuls get no HW speedup and block
FWL weight loads). PSUM accumulation stays fp32, output is fp32.

The host pre-permutes Q's sequence axis to j-major order (q~ = (s%16)*128 +
s//16) so the normalized head outputs land CONTIGUOUSLY in the head/seq-mixed
layout the raw reshape needs -- the DVE normalize writes [64,512] packed
instead of a 16-strided scatter (2.9us -> 0.55us per tile), and the scatter
DMA stays the efficient 256B-run form.

Per-core pipeline:
  1. QpT/KpT = W^T @ x^T   -> [heads*dk, S] layouts (host pre-transposes Q,K,V)
     Vp      = x^T.T @ Wv  -> [S, heads*dk] natural layout, augmented with a
               ones column per head (denominator trick).
  2. Per head pair, per q~ block of 512: per k tile (128 kpos):
     scores^T[s_k, q~] single-bank PSUM tiles (2 heads via row tile_position,
     3-deep lookahead), exp via ACT (scale=1/8 fused, fp32 exp needs no max
     subtraction for |score/8| <~ 25) -> bf16 e tiles, P@V with V_aug
     accumulating [65, 512] PSUM where row 64 is the softmax denominator.
  3. recip (DVE) -> broadcast via K=1 outer-product matmul -> normalize on
     DVE into contiguous hr columns.
  4. One scatter DMA per (head, partition parity) assembles X^T tiles;
     WO matmul per head; DMA out.
"""

import sys

try:
    import concourse.bass as bass  # noqa: F401
except ImportError:
    sys.path.insert(0, "/opt/trn_rl_repo")

import ml_dtypes
import numpy as np

import concourse.bacc as bacc
import concourse.tile as tile
from concourse import mybir
from concourse.bass_utils import run_bass_kernel_spmd

BF16 = mybir.dt.bfloat16
F32 = mybir.dt.float32
NP_BF16 = ml_dtypes.bfloat16

B, S, D, H, DK = 2, 2048, 1024, 16, 64
HEADS_PER_CORE = 4
GROUPS = 4
SCALE = 1.0 / 8.0  # 1/sqrt(dk)
E_BUFS = 16

# j-major permutation of the query/sequence axis: column q~ = j*128 + r of
# the permuted qT holds original sequence position s = r*16 + j.
_J = np.arange(S) // 128
_R = np.arange(S) % 128
Q_PERM = _R * 16 + _J  # q~ -> s

_cached_nc = None


def build_nc():
    nc = bacc.Bacc(None, target_bir_lowering=False)
    qT = nc.dram_tensor("qT", [D, S], BF16, kind="ExternalInput")
    kT = nc.dram_tensor("kT", [D, S], BF16, kind="ExternalInput")
    vT = nc.dram_tensor("vT", [D, S], BF16, kind="ExternalInput")
    wq = nc.dram_tensor("wq", [D, 256], BF16, kind="ExternalInput")
    wk = nc.dram_tensor("wk", [D, 256], BF16, kind="ExternalInput")
    wv = nc.dram_tensor("wv", [D, 256], BF16, kind="ExternalInput")
    wo = nc.dram_tensor("wo", [D, D], BF16, kind="ExternalInput")
    out = nc.dram_tensor("out", [512, D], F32, kind="ExternalOutput")

    Exp = mybir.ActivationFunctionType.Exp

    with tile.TileContext(nc) as tc, nc.allow_low_precision(
        reason="bf16 matmul operands with fp32 PSUM accumulation; attention "
        "weights in bf16 average out over 2048 positions"
    ):
        with (
            tc.tile_pool(name="persist", bufs=1) as persist,
            tc.tile_pool(name="hrp", bufs=2) as hrp,
            tc.tile_pool(name="xhp", bufs=2) as xhp,
            tc.tile_pool(name="small", bufs=3) as small,
            tc.tile_pool(name="pvsb", bufs=4) as pvsb,
            tc.tile_pool(name="opool", bufs=2) as opool,
            tc.tile_pool(name="epool", bufs=E_BUFS) as epool,
            tc.tile_pool(name="ps_mix", bufs=2, space="PSUM") as ps_mix,
            tc.tile_pool(name="ps_sc", bufs=4, space="PSUM") as ps_sc,
            tc.tile_pool(name="ps_pv", bufs=2, space="PSUM") as ps_pv,
        ):
            qpT = persist.tile([128, 2, S], BF16, tag="qpT")
            kpT = persist.tile([128, 2, S], BF16, tag="kpT")
            vaug = persist.tile([128, 16, 4, 65], BF16, tag="vaug")
            ones_f32 = persist.tile([128, 1], F32, tag="ones_f32")
            nc.vector.memset(ones_f32, 1.0)
            nc.vector.tensor_copy(
                vaug[:, :, :, 64:65], ones_f32.to_broadcast((128, 16, 4, 1))
            )
            ones = persist.tile([1, 64], BF16, tag="ones")
            nc.vector.tensor_copy(ones, ones_f32[0:1, :].to_broadcast((1, 64)))

            # ---------------- Phase A: projections ----------------
            # Emission order sets scheduler priority: K first (scores lhsT
            # needs all of it), then Q block 0 (first scores rhs), then V
            # (P@V), then remaining Q blocks, then WO weights.
            with (
                tc.tile_pool(name="wqkv", bufs=1) as wqkv,
                tc.tile_pool(name="stream", bufs=3) as stream,
            ):
                wq_sb = wqkv.tile([128, 8, 256], BF16, tag="wq")
                wk_sb = wqkv.tile([128, 8, 256], BF16, tag="wk")
                wv_sb = wqkv.tile([128, 8, 256], BF16, tag="wv")
                for w_dram, w_sb in ((wk, wk_sb), (wq, wq_sb), (wv, wv_sb)):
                    nc.sync.dma_start(
                        out=w_sb, in_=w_dram.rearrange("(t p) n -> p t n", p=128)
                    )

                def proj_qk(x_dram, w_sb, outt, nb):
                    st = stream.tile([128, 8, 512], BF16, tag="acts", name="stq")
                    nc.sync.dma_start(
                        out=st,
                        in_=x_dram.rearrange("(t p) s -> p t s", p=128)[
                            :, :, 512 * nb : 512 * (nb + 1)
                        ],
                    )
                    for m in range(2):
                        ps = ps_mix.tile([128, 512], F32, tag="mix", name="psq")
                        for k in range(8):
                            nc.tensor.matmul(
                                ps,
                                w_sb[:, k, 128 * m : 128 * (m + 1)],
                                st[:, k, :],
                                start=(k == 0),
                                stop=(k == 7),
                            )
                        nc.vector.tensor_copy(
                            outt[:, m, 512 * nb : 512 * (nb + 1)], ps
                        )

                def proj_v(nb):
                    st = stream.tile([128, 8, 512], BF16, tag="acts", name="stv")
                    nc.sync.dma_start(
                        out=st,
                        in_=vT.rearrange("(t p) s -> p t s", p=128)[
                            :, :, 512 * nb : 512 * (nb + 1)
                        ],
                    )
                    for sti in range(4):
                        stt = 4 * nb + sti
                        ps_full = ps_mix.tile([128, 512], F32, tag="mix", name="vps")
                        ps = ps_full[:, :256]
                        for k in range(8):
                            nc.tensor.matmul(
                                ps,
                                st[:, k, 128 * sti : 128 * (sti + 1)],
                                wv_sb[:, k, :],
                                start=(k == 0),
                                stop=(k == 7),
                            )
                        nc.vector.tensor_copy(
                            vaug[:, stt, :, 0:64],
                            ps.rearrange("p (h c) -> p h c", h=4),
                        )

                for nb in range(4):
                    proj_qk(kT, wk_sb, kpT, nb)
                proj_qk(qT, wq_sb, qpT, 0)
                for nb in range(4):
                    proj_v(nb)
                for nb in range(1, 4):
                    proj_qk(qT, wq_sb, qpT, nb)

            # ------------- Phase B/C: attention + output projection -------------
            with tc.tile_pool(name="wop", bufs=1) as wop:
                wo_sb = wop.tile([128, 8, D], BF16, tag="wo")
                nc.sync.dma_start(
                    out=wo_sb, in_=wo.rearrange("(t p) n -> p t n", p=128)
                )

                def attention_block(hp, qb, hr):
                    """scores -> exp -> PV for one (head-pair, q~ block); returns
                    SBUF copies of the PV tiles (pv PSUM banks release fast)."""
                    hA, hB = 2 * hp, 2 * hp + 1
                    pv = {
                        h: ps_pv.tile([65, 512], F32, tag="pv", name=f"pv{h}")
                        for h in (hA, hB)
                    }
                    for kt in range(16):
                        sc = {}
                        e = {}
                        for i, h in enumerate((hA, hB)):
                            sc[h] = ps_sc.tile([128, 512], F32, tag="sc",
                                               name=f"sc{h}")
                            nc.tensor.matmul(
                                sc[h],
                                kpT[64 * i : 64 * (i + 1), hp,
                                    128 * kt : 128 * (kt + 1)],
                                qpT[64 * i : 64 * (i + 1), hp,
                                    512 * qb : 512 * (qb + 1)],
                                start=True,
                                stop=True,
                                tile_position=(64 * i, 0),
                            )
                        for h in (hA, hB):
                            e[h] = epool.tile([128, 512], BF16, tag="e",
                                              name=f"e{h}")
                            nc.scalar.activation(e[h], sc[h], Exp, scale=SCALE)
                        for h in (hA, hB):
                            nc.tensor.matmul(
                                pv[h],
                                vaug[:, kt, h, :],
                                e[h],
                                start=(kt == 0),
                                stop=(kt == 15),
                            )
                    return pv

                def normalize_block(hp, qb, hr, pv_sb):
                    """reciprocal (DVE) -> partition broadcast (GpSimd, idle
                    engine) -> scale (DVE). No PE involvement, so the PE
                    stream never stalls on the reciprocal latency."""
                    for h in (2 * hp, 2 * hp + 1):
                        rc = small.tile([1, 512], F32, tag="rc", name=f"rc{h}")
                        nc.vector.reciprocal(rc, pv_sb[h][64:65, :])
                        bc_sb = small.tile([64, 512], F32, tag="bcs",
                                           name=f"bcs{h}")
                        nc.gpsimd.partition_broadcast(bc_sb, rc)
                        nc.vector.tensor_mul(
                            hr[h][:, 512 * qb : 512 * (qb + 1)],
                            pv_sb[h][0:64, :], bc_sb,
                        )

                def wo_block(hp, hr):
                    for h in (2 * hp, 2 * hp + 1):
                        xh = xhp.tile([128, 8, 128], BF16, tag="xh", name=f"xh{h}")
                        hv = hr[h].rearrange("p (j r) -> p j r", j=16)
                        for par in range(2):
                            nc.sync.dma_start(
                                out=xh[64 * par : 64 * (par + 1)],
                                in_=hv[:, par::2, :],
                            )
                        for n in range(2):
                            wops = ps_mix.tile([128, 512], F32, tag="mix",
                                               name=f"wops{h}")
                            for t in range(8):
                                nc.tensor.matmul(
                                    wops,
                                    xh[:, t, :],
                                    wo_sb[:, t, 512 * n : 512 * (n + 1)],
                                    start=(t == 0),
                                    stop=(t == 7),
                                )
                            ot = opool.tile([128, 512], F32, tag="o", name=f"ot{h}")
                            nc.vector.tensor_copy(ot, wops)
                            nc.sync.dma_start(
                                out=out[128 * h : 128 * (h + 1),
                                        512 * n : 512 * (n + 1)],
                                in_=ot,
                            )

                hrs = {}
                for hp in range(2):
                    hrs[hp] = {
                        h: hrp.tile([64, 2048], BF16, tag="hr", name=f"hr{h}")
                        for h in (2 * hp, 2 * hp + 1)
                    }
                blocks = [(hp, qb) for hp in range(2) for qb in range(4)]
                for idx, (hp, qb) in enumerate(blocks):
                    pv = attention_block(hp, qb, hrs[hp])
                    # copy PV out of PSUM right away to free the banks
                    pv_sb = {}
                    for h in (2 * hp, 2 * hp + 1):
                        pv_sb[h] = pvsb.tile([65, 512], F32, tag="pvs",
                                             name=f"pvs{h}")
                        nc.vector.tensor_copy(pv_sb[h], pv[h])
                    normalize_block(hp, qb, hrs[hp], pv_sb)
                    if hp == 0 and qb == 3:
                        wo_block(0, hrs[0])
                wo_block(1, hrs[1])

    nc.finalize()
    return nc


def make_in_maps(Q, K, V, WQ, WK, WV, WO):
    in_maps = []
    wo_full = np.ascontiguousarray(WO.astype(NP_BF16))
    for b in range(B):
        # qT columns permuted to j-major order (see Q_PERM)
        qTb = np.ascontiguousarray(Q[b].T[:, Q_PERM].astype(NP_BF16))
        kTb = np.ascontiguousarray(K[b].T.astype(NP_BF16))
        vTb = np.ascontiguousarray(V[b].T.astype(NP_BF16))
        for g in range(GROUPS):
            hs = slice(4 * g, 4 * g + 4)
            # [4, D, dk] -> [D, 4*dk]
            wqc = np.ascontiguousarray(
                WQ[hs].transpose(1, 0, 2).reshape(D, 256).astype(NP_BF16)
            )
            wkc = np.ascontiguousarray(
                WK[hs].transpose(1, 0, 2).reshape(D, 256).astype(NP_BF16)
            )
            wvc = np.ascontiguousarray(
                WV[hs].transpose(1, 0, 2).reshape(D, 256).astype(NP_BF16)
            )
            in_maps.append(
                {"qT": qTb, "kT": kTb, "vT": vTb,
                 "wq": wqc, "wk": wkc, "wv": wvc, "wo": wo_full}
            )
    return in_maps


def run(inputs, **run_kwargs):
    global _cached_nc
    if _cached_nc is None:
        _cached_nc = build_nc()
    in_maps = make_in_maps(**inputs)
    res = run_bass_kernel_spmd(
        _cached_nc, in_maps, core_ids=list(range(8)), **run_kwargs
    )
    full = np.zeros((B, S, D), np.float32)
    for b in range(B):
        for g in range(GROUPS):
            full[b, 512 * g : 512 * (g + 1), :] = res.results[4 * b + g]["out"]
    return full, res


def kernel(**inputs):
    full, _ = run(inputs)
    return full


if __name__ == "__main__":
    rng = np.random.default_rng(0)
    inputs = {
        "Q": rng.standard_normal((B, S, D)).astype(np.float32),
        "K": rng.standard_normal((B, S, D)).astype(np.float32),
        "V": rng.standard_normal((B, S, D)).astype(np.float32),
        "WQ": (rng.uniform(-0.1, 0.1, (H, D, DK))).astype(np.float32),
        "WK": (rng.uniform(-0.1, 0.1, (H, D, DK))).astype(np.float32),
        "WV": (rng.uniform(-0.1, 0.1, (H, D, DK))).astype(np.float32),
        "WO": (rng.uniform(-0.1, 0.1, (H * DK, D))).astype(np.float32),
    }
    out = kernel(**inputs)
    print("kernel out", out.shape, out.dtype, float(np.abs(out).max()))
